# revision 1
# baseline (speedup 1.0000x reference)
"""Causal bag-of-words kernel for Trainium2 (8 NeuronCores, SPMD).

out[b, t, :] = mean(x[b, :t+1, :], axis=0)  for x of shape (8, 8192, 512) f32.

Sharding: data-parallel over B — core b handles x[b] (8192, 512) independently.

Per-core algorithm (all in natural [t, c] layout, no transposes):
  T = 8192 is split into 64 blocks of 128 rows (partition dim).
  For block k with rows X_k [128, 512]:
    psum_k = U @ X_k + J @ Z_{k-1}      (two accumulating PE matmuls)
  where U is upper-triangular ones (cumsum within the block), J is all-ones
  (broadcasts the column-sum of Z over all 128 rows), and
  Z_{k-1} = sum_{j<k} X_j is a running elementwise block sum maintained with
  one DVE add per block.  The carry matmul runs at full PE rate as two exact
  fp32r (TF32-like) matmuls on the hi/lo split of Z (zh = round_fp32r(Z),
  zl = round_fp32r(Z - zh), Z = zh + zl to ~2^-23), computed on ACT/GPSIMD.
  The 1/(t+1) scaling is folded into the PSUM->SBUF evacuation (DVE).
  Blocks stream in waves of 8 (2 MiB DMAs) and are written back the same way.
"""

import sys

sys.path.insert(0, "/opt/trn_rl_repo")

import numpy as np

import concourse.bacc as bacc
import concourse.bass as bass
import concourse.mybir as mybir
import concourse.tile as tile
from concourse.bass_utils import run_bass_kernel_spmd

B, T, C = 8, 8192, 512
P = 128                 # partition dim / block size along T
NB = T // P             # 64 blocks
G = 8                   # blocks per wave (2 MiB per DMA)
NW = NB // G            # 8 waves
N_CORES = 8
F32 = mybir.dt.float32
F32R = mybir.dt.float32r  # full-rate fp32 matmul path (4x faster at N>=256)

_cache: dict = {}


def build_program(n_iter: int = 1, loop_n: int = 1, sub_engine: str = "alt21",
                  carry_mode: str = "hilo", g: int = 8, hoist: bool = False,
                  xin_bufs: int = 4, pair: bool = False,
                  cast_engine: str = "scalar", evac_engine: str = "vector",
                  zbufs: int = 8, zsbufs: int = 4, obufs: int = 3):
    """Build + compile the per-core Bass program (SPMD, identical on all cores).

    n_iter > 1 unrolls the whole computation; loop_n > 1 wraps it in a
    hardware For_i loop (both for timing by the slope method); results are
    identical for any value.  sub_engine: which engine computes zl = z - zh.
    carry_mode: 'hilo' (2 fp32r MMs) or 'fp32' (1 fp32 MM).
    """
    G = g
    NW = NB // G
    nc = bacc.Bacc("TRN2", target_bir_lowering=False, debug=False,
                   num_devices=N_CORES)

    x_d = nc.dram_tensor("x", [T, C], F32, kind="ExternalInput")
    u_d = nc.dram_tensor("u", [P, P], F32, kind="ExternalInput")
    j_d = nc.dram_tensor("jm", [P, P], F32, kind="ExternalInput")
    r_d = nc.dram_tensor("recip", [P, NB], F32, kind="ExternalInput")
    o_d = nc.dram_tensor("out", [T, C], F32, kind="ExternalOutput")

    ACT_COPY = mybir.ActivationFunctionType.Copy
    with tile.TileContext(nc) as tc:
        with (
            tc.tile_pool(name="consts", bufs=1) as consts,
            tc.tile_pool(name="xin", bufs=xin_bufs) as xin,
            tc.tile_pool(name="oput", bufs=obufs) as oput,
            tc.tile_pool(name="zp", bufs=zbufs if not pair else 5) as zp,
            tc.tile_pool(name="zhp", bufs=zsbufs if not pair else 3) as zhp,
            tc.tile_pool(name="zlp", bufs=zsbufs if not pair else 3) as zlp,
            tc.tile_pool(name="ps", bufs=8 if not pair else 4,
                         space="PSUM") as psp,
        ):
            # consts go via SWDGE (gpsimd) so the HWDGE rings start on the
            # first wave load immediately
            u_t = consts.tile([P, P], F32)
            j_t = consts.tile([P, P], F32)
            r_t = consts.tile([P, NB], F32)
            nc.gpsimd.dma_start(u_t[:], u_d[:])
            nc.gpsimd.dma_start(j_t[:], j_d[:])
            nc.gpsimd.dma_start(r_t[:], r_d[:])
            # ones matrix rounded to fp32r (exact) for the full-rate carry MMs
            j_r = consts.tile([P, P], F32R)
            nc.vector.tensor_copy(j_r[:], j_t[:])

            from contextlib import ExitStack
            loop_ctx = ExitStack()
            if loop_n > 1:
                loop_ctx.enter_context(tc.For_i(0, loop_n, 1))
            H = G // 2          # half-wave (1 MiB DMA granularity)
            for _ in range(n_iter):
                # pass 1: input DMAs + the z-chain, emitted first so the
                # chain adds outrank the evacuation copies on DVE's queue
                def emit_load(w):
                    xw = xin.tile([P, G, C], F32, tag="xw", name=f"xw{w}_")
                    xv = x_d[w * G * P:(w + 1) * G * P, :].rearrange(
                        "(j p) c -> p j c", p=P)
                    if w == 0:
                        # split the first load so PE starts sooner
                        for q in range(G // 2):
                            nc.sync.dma_start(xw[:, 2 * q:2 * q + 2, :],
                                              xv[:, 2 * q:2 * q + 2, :])
                    else:
                        nc.sync.dma_start(xw[:], xv)
                    return xw
                xws = [emit_load(w) for w in range(NW)] if hoist else []
                zs = [None]
                z_prev = None
                if hoist:
                    for k in range(NB - 1):
                        z_new = zp.tile([P, C], F32, tag="z", name=f"z{k}")
                        xk = xws[k // G][:, k % G, :]
                        if k == 0:
                            nc.vector.tensor_copy(z_new[:], xk)
                        else:
                            nc.vector.tensor_add(z_new[:], z_prev[:], xk)
                        zs.append(z_new)
                        z_prev = z_new
                # pass 2: splits, matmuls, evacuations, stores
                if pair:
                    zps = [zp.tile([P, 2, C], F32, tag="z", name=f"zpr{p2}")
                           for p2 in range(NB // 2)]
                    zhps = [None] * (NB // 2)
                    zlps = [None] * (NB // 2)
                    z_prev = None
                for w in range(NW):
                    xw = xws[w] if hoist else emit_load(w)
                    ow = oput.tile([P, G, C], F32, tag="ow")
                    if carry_mode == "hilo_const":
                        zh_diag = zhp.tile([P, C], F32R, tag="zh")
                        nc.scalar.activation(zh_diag[:], xw[:, 0, :], ACT_COPY)
                    if pair:
                        for j in range(G):
                            k = w * G + j
                            xk = xw[:, j, :]
                            # z-chain into pair slots (before the split reads)
                            if k < NB - 1:
                                zpt = zps[k // 2]
                                if k == 0:
                                    nc.vector.tensor_copy(zpt[:, 0, :], xk)
                                else:
                                    nc.vector.tensor_add(
                                        zpt[:, k % 2, :], z_prev, xk)
                                z_prev = zpt[:, k % 2, :]
                            # pair-granular split: one ACT cast + one GPSIMD
                            # sub cover the carries of blocks k and k+1
                            if k % 2 == 1:
                                p2 = k // 2
                                wdt = 2 if k < NB - 1 else 1
                                zpt = zps[p2]
                                zhp_t = zhp.tile([P, 2, C], F32R, tag="zh",
                                                 name=f"zh{p2}")
                                zlp_t = zlp.tile([P, 2, C], F32R, tag="zl",
                                                 name=f"zl{p2}")
                                zhps[p2] = zhp_t
                                zlps[p2] = zlp_t
                                nc.scalar.activation(
                                    zhp_t[:, :wdt, :], zpt[:, :wdt, :],
                                    ACT_COPY)
                                sub_eng = getattr(nc, sub_engine)
                                sub_eng.tensor_sub(
                                    zlp_t[:, :wdt, :], zpt[:, :wdt, :],
                                    zhp_t[:, :wdt, :].bitcast(F32))
                            if k % 2 == 0:
                                pspair = psp.tile([P, 2, C], F32, tag="ps")
                            psv = pspair[:, k % 2, :]
                            if k == 0:
                                nc.tensor.matmul(psv, u_t[:], xk,
                                                 start=True, stop=True)
                            else:
                                p_idx = (k - 1) // 2
                                m = (k - 1) % 2
                                nc.tensor.matmul(psv, u_t[:], xk,
                                                 start=True, stop=False)
                                nc.tensor.matmul(psv, j_r[:],
                                                 zhps[p_idx][:, m, :],
                                                 start=False, stop=False)
                                nc.tensor.matmul(psv, j_r[:],
                                                 zlps[p_idx][:, m, :],
                                                 start=False, stop=True)
                            # paired evacuation with broadcast recip
                            if k % 2 == 1:
                                rb = r_t[:, k - 1:k + 1].rearrange(
                                    "p (b o) -> p b o", o=1).broadcast_to(
                                    [P, 2, C])
                                nc.vector.tensor_tensor(
                                    ow[:, j - 1:j + 1, :], pspair[:], rb,
                                    op=mybir.AluOpType.mult)
                        ov = o_d[w * G * P:(w + 1) * G * P, :].rearrange(
                            "(j p) c -> p j c", p=P)
                        if w == NW - 1:
                            for q in range(G // 2):
                                nc.scalar.dma_start(
                                    ov[:, 2 * q:2 * q + 2, :],
                                    ow[:, 2 * q:2 * q + 2, :])
                        else:
                            nc.scalar.dma_start(ov[:, :H, :], ow[:, :H, :])
                            nc.scalar.dma_start(ov[:, H:, :], ow[:, H:, :])
                        continue
                    for j in range(G):
                        k = w * G + j
                        xk = xw[:, j, :]
                        if hoist:
                            z_prev = zs[k]
                        ps = psp.tile([P, C], F32, tag="ps")
                        if k == 0 or carry_mode == "none":
                            nc.tensor.matmul(ps[:], u_t[:], xk,
                                             start=True, stop=True)
                        elif carry_mode == "hilo_const":
                            # diagnostic: carry MMs read a per-wave const tile
                            # (wrong results; isolates PE cost from the chain)
                            nc.tensor.matmul(ps[:], u_t[:], xk,
                                             start=True, stop=False)
                            nc.tensor.matmul(ps[:], j_r[:], zh_diag[:],
                                             start=False, stop=False)
                            nc.tensor.matmul(ps[:], j_r[:], zh_diag[:],
                                             start=False, stop=True)
                        elif carry_mode == "fp32":
                            nc.tensor.matmul(ps[:], u_t[:], xk,
                                             start=True, stop=False)
                            nc.tensor.matmul(ps[:], j_t[:], z_prev[:],
                                             start=False, stop=True)
                        else:
                            # carry = J @ Z_{k-1}, exact via fp32r hi+lo:
                            # zh = round_fp32r(z), zl = round_fp32r(z - zh)
                            zh = zhp.tile([P, C], F32R, tag="zh")
                            if cast_engine == "scalar":
                                nc.scalar.activation(zh[:], z_prev[:],
                                                     ACT_COPY)
                            else:
                                getattr(nc, cast_engine).tensor_copy(
                                    zh[:], z_prev[:])
                            zl = zlp.tile([P, C], F32R, tag="zl")
                            if sub_engine == "alt":
                                sub_eng = nc.gpsimd if k % 2 else nc.vector
                            elif sub_engine == "alt21":
                                sub_eng = nc.vector if k % 3 == 0 else nc.gpsimd
                            else:
                                sub_eng = getattr(nc, sub_engine)
                            sub_eng.tensor_sub(zl[:], z_prev[:],
                                               zh[:].bitcast(F32))
                            nc.tensor.matmul(ps[:], u_t[:], xk,
                                             start=True, stop=False)
                            nc.tensor.matmul(ps[:], j_r[:], zh[:],
                                             start=False, stop=False)
                            nc.tensor.matmul(ps[:], j_r[:], zl[:],
                                             start=False, stop=True)
                        if not hoist and k < NB - 1:
                            z_new = zp.tile([P, C], F32, tag="z")
                            if k == 0:
                                nc.vector.tensor_copy(z_new[:], xk)
                            else:
                                nc.vector.tensor_add(z_new[:], z_prev[:], xk)
                            zs.append(z_new)
                            z_prev = z_new
                        # evacuate PSUM with the 1/(t+1) scale
                        if evac_engine == "vector":
                            nc.vector.tensor_scalar_mul(
                                ow[:, j, :], ps[:], r_t[:, k:k + 1])
                        else:
                            nc.scalar.activation(
                                ow[:, j, :], ps[:], ACT_COPY,
                                scale=r_t[:, k:k + 1])
                    ov = o_d[w * G * P:(w + 1) * G * P, :].rearrange(
                        "(j p) c -> p j c", p=P)
                    # store per half-wave so the DMA starts 4 blocks earlier;
                    # quarter the final stores to shorten the drain
                    if w == NW - 1:
                        for q in range(G // 2):
                            nc.scalar.dma_start(ov[:, 2 * q:2 * q + 2, :],
                                                ow[:, 2 * q:2 * q + 2, :])
                    else:
                        nc.scalar.dma_start(ov[:, :H, :], ow[:, :H, :])
                        nc.scalar.dma_start(ov[:, H:, :], ow[:, H:, :])
            loop_ctx.close()

    nc.compile()
    return nc


def make_consts():
    s = np.arange(P)
    u = (s[:, None] <= s[None, :]).astype(np.float32)          # u[s,t]=1 if s<=t
    jm = np.ones((P, P), dtype=np.float32)
    counts = (np.arange(NB)[None, :] * P + s[:, None] + 1)     # [P, NB]
    recip = (1.0 / counts).astype(np.float32)
    return u, jm, recip


def kernel(x):
    x = np.ascontiguousarray(np.asarray(x), dtype=np.float32)
    assert x.shape == (B, T, C), x.shape
    if "nc" not in _cache:
        _cache["nc"] = build_program()
    nc = _cache["nc"]
    u, jm, recip = make_consts()
    in_maps = [{"x": x[b], "u": u, "jm": jm, "recip": recip}
               for b in range(N_CORES)]
    res = run_bass_kernel_spmd(nc, in_maps, list(range(N_CORES)))
    out = np.stack([res.results[b]["out"] for b in range(N_CORES)], axis=0)
    return out.astype(np.float32, copy=False)

